# revision 13
# baseline (speedup 1.0000x reference)
# Trainium2 Bass kernel for nn_MemoryRetrieval (spiking memory-retrieval block).
# Data-parallel over batch B=16 across 8 cores (2 batch elements per core).
# All weight matmuls: fp32r hi/lo 2-pass (~fp32-exact); spikes/kv are exact in fp32r.
import sys
sys.path.insert(0, '/opt/trn_rl_repo')
from contextlib import ExitStack

import numpy as np

import concourse.bass as bass
import concourse.tile as tile
import concourse.mybir as mybir
from concourse import bacc
from concourse.bass_utils import run_bass_kernel_spmd
from concourse.masks import make_identity

F32 = mybir.dt.float32
F32R = mybir.dt.float32r
BF16 = mybir.dt.bfloat16
F16 = mybir.dt.float16
ALU = mybir.AluOpType
ACTF = mybir.ActivationFunctionType

DECAY = 0.25
THRESH = 0.5
DIM, HEADS, EXP, PATCH = 384, 8, 1536, 24
N = PATCH * PATCH            # 576
CH = DIM // HEADS            # 48
DH = EXP // HEADS            # 192
SCALE2 = 2.0 * (CH ** -0.5)  # attn scale*2 folded into LIF threshold
THR_ATTN = THRESH / SCALE2
NB = 2
T = 4
KC_C = DIM // 128            # 3
KC_E = EXP // 128            # 12
NCH = [(0, 128), (128, 128), (256, 128), (384, 128), (512, 64)]
NHS = [(0, 288), (288, 288)]

DEBUG_TAPS = False


def _round12(a):
    a = np.asarray(a, np.float32)
    m, e = np.frexp(a)
    scale = np.ldexp(np.float32(1.0), (12 - e.astype(np.int32)))
    with np.errstate(invalid='ignore'):
        r = (np.round(a.astype(np.float64) * scale) / scale).astype(np.float32)
    return np.where(a == 0.0, np.float32(0.0), r)


def _hilo(w):
    hi = _round12(w)
    lo = _round12((w - hi).astype(np.float32))
    return np.ascontiguousarray(hi), np.ascontiguousarray(lo)


def _prep_weights(p):
    d = {}

    def conv_w(name, w, s):
        wT = (w * s[:, None]).T.astype(np.float32)  # [Cin, Cout]
        hi, lo = _hilo(wT)
        d[f'{name}_hi'], d[f'{name}_lo'] = hi, lo

    conv_w('wk', p['wk'], p['sk'])
    conv_w('wv', p['wv'], p['sv'])
    conv_w('wq', p['wq'], p['sq'])
    conv_w('wfuse', p['wfuse'], p['sfuse'])
    conv_w('wproj', p['wproj'], p['sproj'])
    conv_w('wfc1', p['wfc1'], p['sfc1'])
    conv_w('wfc2', p['wfc2'], p['sfc2'])
    for t in range(T):
        conv_w(f'wpw{t}', p['wpw'][t], p['spw'][t])
    hi, lo = _hilo((p['wgate'] / float(N)).astype(np.float32))
    d['wgate_hi'], d['wgate_lo'] = hi, lo
    wdw = (p['wdw'][:, :, 0] * p['sdw'][:, :, None, None]).astype(np.float32)
    d['wdw'] = np.ascontiguousarray(
        wdw.reshape(T, KC_E, 128, 9).transpose(0, 2, 1, 3).reshape(T, 128, KC_E * 9))
    for nm in ('bk', 'bv', 'bq', 'bdw', 'bpw', 'bfuse', 'bproj', 'bfc1', 'bfc2'):
        if np.any(np.asarray(p[nm])):
            raise NotImplementedError(f'nonzero bias {nm} not supported')
    return d


_BUILT = None


def _remap_pad_dmas(nc, dst_pad, src_cmaj):
    """scatter [384ch Cmaj] -> padded-head [128, 4, N] (head h -> tile h//2, rows 64*(h%2))."""
    for h in range(HEADS):
        r0 = h * CH
        dst0 = 64 * (h % 2)
        while r0 < (h + 1) * CH:
            kc, rr = divmod(r0, 128)
            span = min((h + 1) * CH - r0, 128 - rr)
            o = r0 - h * CH
            nc.sync.dma_start(dst_pad[dst0 + o: dst0 + o + span, h // 2, :],
                              src_cmaj[rr:rr + span, kc, :])
            r0 += span


def _build():
    nc = bacc.Bacc("TRN2", target_bir_lowering=False, debug=False)

    templ_d = nc.dram_tensor("templates", [T, NB, DIM, N], F32, kind="ExternalInput").ap()
    search_d = nc.dram_tensor("search", [NB, DIM, N], F32, kind="ExternalInput").ap()
    srpos_d = nc.dram_tensor("search_pos", [NB, DIM, N], F32, kind="ExternalInput").ap()
    w_d = {}
    for nm, cin, cout in (('wk', DIM, DIM), ('wv', DIM, EXP), ('wq', DIM, DIM),
                          ('wfuse', DIM, DIM), ('wproj', EXP, DIM),
                          ('wfc1', DIM, EXP), ('wfc2', EXP, DIM), ('wgate', EXP, EXP),
                          *[(f'wpw{t}', EXP, DIM) for t in range(T)]):
        for hl in ('hi', 'lo'):
            w_d[f'{nm}_{hl}'] = nc.dram_tensor(f'{nm}_{hl}', [cin, cout], F32R,
                                               kind="ExternalInput").ap()
    wdw_d = nc.dram_tensor("wdw", [T, 128, KC_E * 9], F32, kind="ExternalInput").ap()
    zeros_d = nc.dram_tensor("zeros16", [16, N], F16, kind="ExternalInput").ap()
    out_d = nc.dram_tensor("out", [NB, DIM, N], F32, kind="ExternalOutput").ap()

    with tile.TileContext(nc) as tc, ExitStack() as octx:
        st = octx.enter_context(tc.tile_pool(name="carry", bufs=1))
        dr = octx.enter_context(tc.tile_pool(name="dram", bufs=1, space="DRAM"))

        ident = st.tile([128, 128], F32, name="ident")
        make_identity(nc, ident[:])

        def load_w(pool, nm, kc, tag):
            ap = w_d[nm]
            tl = pool.tile([128, ap.shape[1]], F32R, tag=tag, name=f"w_{nm}_{kc}")
            nc.sync.dma_start(tl[:], ap[kc * 128:(kc + 1) * 128, :])
            return tl

        def lif_step(t, mem, g, x_ap, spike, thr=THRESH, accum=None, sdt=None):
            if t == 0:
                nc.vector.tensor_scalar(mem, x_ap, 1.0, None, ALU.mult)
            else:
                nc.vector.scalar_tensor_tensor(g, mem, thr, mem, ALU.is_le, ALU.mult)
                nc.vector.scalar_tensor_tensor(mem, g, DECAY, x_ap, ALU.mult, ALU.add)
            if spike is not None:
                nc.vector.tensor_scalar(spike, mem, thr, None, ALU.is_gt)
                if accum is not None:
                    nc.vector.tensor_reduce(accum, spike, mybir.AxisListType.X,
                                            ALU.add)

        for b in range(NB):
            kv_pad = st.tile([128, T, 4, DH], F16, name="kv_pad")
            sc_cmaj = st.tile([128, KC_C, N], F32, name="sc_cmaj")
            qs_pad = st.tile([128, 4, N], F32, name="qs_pad")
            pooled = st.tile([128, KC_E, T], F32, name="pooled")
            s_dram = dr.tile([T, KC_E, 128, N], BF16, name="s_dram")
            fu_dram = dr.tile([T, KC_C, 128, N], F16, name="fu_dram")
            nc.any.memset(qs_pad[:], 0.0)

            # ================= stage A =================
            with ExitStack() as ctx:
                wkp = ctx.enter_context(tc.tile_pool(name="wkA", bufs=2))
                sa = ctx.enter_context(tc.tile_pool(name="sa", bufs=1))
                # --- sr spike + q conv ---
                with ExitStack() as qctx:
                    psq = qctx.enter_context(tc.tile_pool(name="psq", bufs=2, space="PSUM"))
                    xq = sa.tile([128, KC_C, N], F32, name="xq")
                    nc.sync.dma_start(xq[:], srpos_d[b].rearrange("(k p) n -> p k n", p=128))
                    sr_sp = sa.tile([128, KC_C, N], F32R, name="sr_sp")
                    nc.vector.tensor_scalar(sr_sp[:], xq[:], THRESH, None, ALU.is_gt)
                    for mc in range(KC_C):
                        pt = psq.tile([128, 2, 512], F32, tag="mm", name="q_ps")
                        first = True
                        for kc in range(KC_C):
                            for hl in ('hi', 'lo'):
                                wtl = load_w(wkp, f'wq_{hl}', kc, "w_small")
                                for ih, (n0, nsz) in enumerate(NHS):
                                    nc.tensor.matmul(
                                        pt[:, ih, :nsz], wtl[:, mc * 128:(mc + 1) * 128],
                                        sr_sp[:, kc, n0:n0 + nsz],
                                        start=first, stop=(kc == KC_C - 1 and hl == 'lo'))
                                first = False
                        for ih, (n0, nsz) in enumerate(NHS):
                            nc.any.tensor_copy(sc_cmaj[:, mc, n0:n0 + nsz], pt[:, ih, :nsz])
                _remap_pad_dmas(nc, qs_pad, sc_cmaj)

                # --- per-t: tm LIF, k/v transposed convs + LIF, kv ---
                psa = ctx.enter_context(tc.tile_pool(name="psa", bufs=1, space="PSUM"))
                psk = ctx.enter_context(tc.tile_pool(name="psk", bufs=1, space="PSUM"))
                wres = ctx.enter_context(tc.tile_pool(name="wres", bufs=1))
                wk_res = {}
                wv_res = {}
                for kc in range(KC_C):
                    for hl in ('hi', 'lo'):
                        wk_res[kc, hl] = load_w(wres, f'wk_{hl}', kc, f"wkr{kc}{hl}")
                        wv_res[kc, hl] = load_w(wres, f'wv_{hl}', kc, f"wvr{kc}{hl}")
                mem_tm = sa.tile([128, KC_C, N], F32, name="mem_tm")
                g_tm = sa.tile([128, KC_C, N], F32, name="g_tm")
                mem_k = sa.tile([128, 5, DIM], F32, name="mem_k")
                mem_v = sa.tile([128, 5, EXP], F32, name="mem_v")
                for t in range(T):
                    xt = wkp.tile([128, KC_C, N], F32, name="xt")
                    nc.sync.dma_start(xt[:], templ_d[t, b].rearrange("(k p) n -> p k n", p=128))
                    tm_sp = wkp.tile([128, KC_C, N], F32R, name="tm_sp")
                    lif_step(t, mem_tm[:], g_tm[:], xt[:], tm_sp[:])

                    kT_sp = sa.tile([128, 5, DIM], F16, name="kT_sp")
                    kvp = [psk.tile([128, 384], F32, tag=f"kv{p_}", name=f"kvp{p_}")
                           for p_ in range(4)]
                    for p_ in range(4):
                        nc.vector.memset(kvp[p_][:], 0.0)
                    for inc, (n0, nsz) in enumerate(NCH):
                        ktp = psa.tile([128, DIM], F32, tag="p384", name="ktp")
                        vtp = psa.tile([128, EXP], F32, tag="big", name="vtp")
                        first = True
                        for kc in range(KC_C):
                            for hl in ('hi', 'lo'):
                                wk_tl = wk_res[kc, hl]
                                wv_tl = wv_res[kc, hl]
                                lhsT = tm_sp[:, kc, n0:n0 + nsz]
                                last = (kc == KC_C - 1 and hl == 'lo')
                                nc.tensor.matmul(ktp[:nsz, :], lhsT, wk_tl[:, :],
                                                 start=first, stop=last)
                                for j in range(3):
                                    nc.tensor.matmul(vtp[:nsz, j * 512:(j + 1) * 512],
                                                     lhsT, wv_tl[:, j * 512:(j + 1) * 512],
                                                     start=first, stop=last)
                                first = False
                        g_nm = wkp.tile([128, EXP], F32, tag="g_nm", name="g_nm")
                        vT_sp = wkp.tile([128, EXP], F16, tag="vT_sp", name="vT_sp")
                        lif_step(t, mem_k[:nsz, inc, :], g_nm[:nsz, :DIM],
                                 ktp[:nsz, :], kT_sp[:nsz, inc, :])
                        lif_step(t, mem_v[:nsz, inc, :], g_nm[:nsz, :],
                                 vtp[:nsz, :], vT_sp[:nsz, :])
                        st_ = inc == 0
                        sp_ = inc == len(NCH) - 1
                        for p_ in range(4):
                            nc.tensor.matmul(kvp[p_][0:48, :],
                                             kT_sp[:nsz, inc, 96 * p_:96 * p_ + 48],
                                             vT_sp[:nsz, 384 * p_:384 * p_ + 384],
                                             start=st_, stop=sp_)
                            nc.tensor.matmul(kvp[p_][64:112, :],
                                             kT_sp[:nsz, inc, 96 * p_ + 48:96 * p_ + 96],
                                             vT_sp[:nsz, 384 * p_:384 * p_ + 384],
                                             start=st_, stop=sp_)
                    for p_ in range(4):
                        nc.any.tensor_copy(kv_pad[0:64, t, p_, :], kvp[p_][0:64, 0:DH])
                        nc.any.tensor_copy(kv_pad[64:128, t, p_, :], kvp[p_][64:128, DH:2 * DH])

            def attn_mms(t, c, rhs_pad, pt):
                p_, r = divmod(c, 3)
                for ih, (n0, nsz) in enumerate(NHS):
                    rhs_e = rhs_pad[0:64, p_, n0:n0 + nsz]
                    rhs_o = rhs_pad[64:128, p_, n0:n0 + nsz]
                    if r == 0:
                        nc.tensor.matmul(pt[0:128, ih, :nsz],
                                         kv_pad[0:64, t, p_, 0:128], rhs_e)
                    elif r == 1:
                        nc.tensor.matmul(pt[0:64, ih, :nsz],
                                         kv_pad[0:64, t, p_, 128:192], rhs_e)
                        nc.tensor.matmul(pt[64:128, ih, :nsz],
                                         kv_pad[64:128, t, p_, 0:64], rhs_o,
                                         tile_position=(64, 64))
                    else:
                        nc.tensor.matmul(pt[0:128, ih, :nsz],
                                         kv_pad[64:128, t, p_, 64:192], rhs_o,
                                         tile_position=(64, 0))

            # ================= stage B1: attn1 -> dw -> pw -> fuse =================
            with ExitStack() as ctx:
                wkp = ctx.enter_context(tc.tile_pool(name="wkB", bufs=2))
                sb = ctx.enter_context(tc.tile_pool(name="sb1", bufs=1))
                psm = ctx.enter_context(tc.tile_pool(name="psm", bufs=1, space="PSUM"))
                mem_q = sb.tile([128, 4, N], F32, name="mem_q")
                mem_qq = sb.tile([128, KC_E, N], F32, name="mem_qq")
                mem_dw = sb.tile([128, KC_E, N], F32, name="mem_dw")
                mem_pw = sb.tile([128, KC_C, N], F32, name="mem_pw")
                mem_fu = sb.tile([128, KC_C, N], F32, name="mem_fu")
                dw_sp = sb.tile([128, KC_E, N], F32R, name="dw_sp")
                fu_in = sb.tile([128, KC_C, N], F32R, name="fu_in")
                fu_sp = sb.tile([128, KC_C, N], F16, name="fu_sp")
                q_sp = sb.tile([128, 4, N], F16, name="q_sp")
                wdw_sb = sb.tile([128, T, KC_E * 9], F32, name="wdw_sb")
                nc.sync.dma_start(wdw_sb[:], wdw_d.rearrange("t p k -> p t k"))
                for t in range(T):
                    for p_ in range(4):
                        g_c = wkp.tile([128, N], F32, tag="g_c", name="g_cq")
                        lif_step(t, mem_q[:, p_, :], g_c[:], qs_pad[:, p_, :],
                                 q_sp[:, p_, :])
                    for c in range(KC_E):
                        pt = psm.tile([128, 2, 512], F32, tag="mm", name="a1_ps")
                        attn_mms(t, c, q_sp, pt)
                        g_c = wkp.tile([128, N], F32, tag="g_c", name="g_c1")
                        x_view = pt[:, :, 0:288]
                        if t == 0:
                            nc.vector.tensor_scalar(mem_qq[:, c, :], x_view, 1.0,
                                                    None, ALU.mult)
                        else:
                            nc.vector.scalar_tensor_tensor(g_c[:], mem_qq[:, c, :],
                                                           THR_ATTN, mem_qq[:, c, :],
                                                           ALU.is_le, ALU.mult)
                            nc.vector.scalar_tensor_tensor(mem_qq[:, c, :], g_c[:],
                                                           DECAY, x_view, ALU.mult, ALU.add)
                        dpad = wkp.tile([128, 26, 26], F32, tag="dpad", name="dpad")
                        nc.any.memset(dpad[:], 0.0)
                        nc.vector.tensor_scalar(dpad[:, 1:25, 1:25], mem_qq[:, c, :],
                                                THR_ATTN, None, ALU.is_gt)
                        acc_d = wkp.tile([128, 24, 24], F32, tag="dacc", name="dacc")
                        acc_g = wkp.tile([128, 24, 24], F32, tag="daccg", name="daccg")
                        for tap in range(6):
                            dy, dx = divmod(tap, 3)
                            w_ap = wdw_sb[:, t, c * 9 + tap: c * 9 + tap + 1]
                            view = dpad[:, dy:dy + 24, dx:dx + 24]
                            if tap == 0:
                                nc.vector.tensor_scalar(acc_d[:], view, w_ap, None, ALU.mult)
                            else:
                                nc.vector.scalar_tensor_tensor(acc_d[:], view, w_ap,
                                                               acc_d[:], ALU.mult, ALU.add)
                        for tap in range(6, 9):
                            dy, dx = divmod(tap, 3)
                            w_ap = wdw_sb[:, t, c * 9 + tap: c * 9 + tap + 1]
                            view = dpad[:, dy:dy + 24, dx:dx + 24]
                            if tap == 6:
                                nc.scalar.activation(acc_g[:], view, ACTF.Copy,
                                                     scale=w_ap)
                            else:
                                ztap = wkp.tile([128, 24, 24], F32, tag="ztap",
                                                name="ztap")
                                nc.scalar.activation(ztap[:], view, ACTF.Copy,
                                                     scale=w_ap)
                                nc.gpsimd.tensor_tensor(acc_g[:], acc_g[:], ztap[:],
                                                        ALU.add)
                        # dw LIF with two partial accumulators
                        if t == 0:
                            nc.vector.tensor_scalar(mem_dw[:, c, :],
                                                    acc_d[:].rearrange("p a b -> p (a b)"),
                                                    1.0, None, ALU.mult)
                        else:
                            nc.vector.scalar_tensor_tensor(g_c[:], mem_dw[:, c, :],
                                                           THRESH, mem_dw[:, c, :],
                                                           ALU.is_le, ALU.mult)
                            nc.vector.scalar_tensor_tensor(
                                mem_dw[:, c, :], g_c[:], DECAY,
                                acc_d[:].rearrange("p a b -> p (a b)"),
                                ALU.mult, ALU.add)
                        nc.vector.tensor_tensor(mem_dw[:, c, :], mem_dw[:, c, :],
                                                acc_g[:].rearrange("p a b -> p (a b)"),
                                                ALU.add)
                        nc.vector.tensor_scalar(dw_sp[:, c, :], mem_dw[:, c, :],
                                                THRESH, None, ALU.is_gt)

                    pw_pts = [psm.tile([128, 2, 512], F32, tag=f"pw{mc}", name=f"pw_ps{mc}")
                              for mc in range(KC_C)]
                    first = True
                    for kc in range(KC_E):
                        for hl in ('hi', 'lo'):
                            wtl = load_w(wkp, f'wpw{t}_{hl}', kc, "w_small")
                            for mc in range(KC_C):
                                for ih, (n0, nsz) in enumerate(NHS):
                                    nc.tensor.matmul(pw_pts[mc][:, ih, :nsz],
                                                     wtl[:, mc * 128:(mc + 1) * 128],
                                                     dw_sp[:, kc, n0:n0 + nsz],
                                                     start=first,
                                                     stop=(kc == KC_E - 1 and hl == 'lo'))
                            first = False
                    for mc in range(KC_C):
                        pt = pw_pts[mc]
                        g_c = wkp.tile([128, N], F32, tag="g_c", name="g_c2")
                        if t == 0:
                            nc.vector.tensor_scalar(mem_pw[:, mc, :], pt[:, :, 0:288],
                                                    1.0, None, ALU.mult)
                        else:
                            nc.vector.scalar_tensor_tensor(g_c[:], mem_pw[:, mc, :],
                                                           THRESH, mem_pw[:, mc, :],
                                                           ALU.is_le, ALU.mult)
                            nc.vector.scalar_tensor_tensor(mem_pw[:, mc, :], g_c[:],
                                                           DECAY, pt[:, :, 0:288],
                                                           ALU.mult, ALU.add)
                        nc.vector.tensor_tensor(mem_pw[:, mc, :], mem_pw[:, mc, :],
                                                sc_cmaj[:, mc, :], ALU.add)
                        nc.vector.tensor_scalar(fu_in[:, mc, :], mem_pw[:, mc, :],
                                                THRESH, None, ALU.is_gt)

                    fu_pts = [psm.tile([128, 2, 512], F32, tag=f"pw{mc}", name=f"fu_ps{mc}")
                              for mc in range(KC_C)]
                    first = True
                    for kc in range(KC_C):
                        for hl in ('hi', 'lo'):
                            wtl = load_w(wkp, f'wfuse_{hl}', kc, "w_small")
                            for mc in range(KC_C):
                                for ih, (n0, nsz) in enumerate(NHS):
                                    nc.tensor.matmul(fu_pts[mc][:, ih, :nsz],
                                                     wtl[:, mc * 128:(mc + 1) * 128],
                                                     fu_in[:, kc, n0:n0 + nsz],
                                                     start=first,
                                                     stop=(kc == KC_C - 1 and hl == 'lo'))
                            first = False
                    for mc in range(KC_C):
                        pt = fu_pts[mc]
                        g_c = wkp.tile([128, N], F32, tag="g_c", name="g_c3")
                        lif_step(t, mem_fu[:, mc, :], g_c[:], pt[:, :, 0:288],
                                 fu_sp[:, mc, :])
                        nc.sync.dma_start(fu_dram[t, mc], fu_sp[:, mc, :])

            # ================= stage B2: attn2 + gate =================
            with ExitStack() as ctx:
                wkp = ctx.enter_context(tc.tile_pool(name="wkC", bufs=2))
                sb = ctx.enter_context(tc.tile_pool(name="sb2", bufs=1))
                psm = ctx.enter_context(tc.tile_pool(name="psm2", bufs=2, space="PSUM"))
                psg = ctx.enter_context(tc.tile_pool(name="psg", bufs=1, space="PSUM"))
                mem_at = sb.tile([128, KC_E, N], F32, name="mem_at")
                for t in range(T):
                    fu_pad = wkp.tile([128, 4, N], F16, tag="fu_pad", name="fu_pad")
                    for pp in range(4):
                        nc.sync.dma_start(fu_pad[48:64, pp, :], zeros_d)
                        nc.sync.dma_start(fu_pad[112:128, pp, :], zeros_d)
                    for h in range(HEADS):
                        r0 = h * CH
                        dst0 = 64 * (h % 2)
                        while r0 < (h + 1) * CH:
                            kc, rr = divmod(r0, 128)
                            span = min((h + 1) * CH - r0, 128 - rr)
                            o = r0 - h * CH
                            nc.sync.dma_start(
                                fu_pad[dst0 + o:dst0 + o + span, h // 2, :],
                                fu_dram[t, kc, rr:rr + span, :])
                            r0 += span
                    for c in range(KC_E):
                        pt = psm.tile([128, 2, 512], F32, tag="mm", name="a2_ps")
                        attn_mms(t, c, fu_pad, pt)
                        g_c = wkp.tile([128, N], F32, tag="g_c", name="g_c4")
                        s_t = wkp.tile([128, N], BF16, tag="s_t", name="s_t")
                        lif_step(t, mem_at[:, c, :], g_c[:], pt[:, :, 0:288],
                                 s_t[:], thr=THR_ATTN, accum=pooled[:, c, t:t + 1])
                        nc.sync.dma_start(s_dram[t, c], s_t[:])
                # gate weights
                pooled_r = sb.tile([128, KC_E, T], F32R, name="pooled_r")
                nc.any.tensor_copy(pooled_r[:], pooled[:])
                gl_ps = psg.tile([4, EXP], F32, tag="gl", name="gl_ps")
                first = True
                for kc in range(KC_E):
                    for hl in ('hi', 'lo'):
                        wtl = load_w(wkp, f'wgate_{hl}', kc, "w_big")
                        for j in range(3):
                            nc.tensor.matmul(gl_ps[:, j * 512:(j + 1) * 512],
                                             pooled_r[:, kc, :],
                                             wtl[:, j * 512:(j + 1) * 512],
                                             start=first, stop=(kc == KC_E - 1 and hl == 'lo'))
                        first = False
                gl_sb = sb.tile([4, EXP], F32, name="gl_sb")
                nc.any.tensor_copy(gl_sb[:], gl_ps[:])
                gw = st.tile([128, KC_E, T], F32, name="gw")
                for c in range(KC_E):
                    gt_ps = psg.tile([128, 4], F32, tag="gt", name="gt_ps")
                    nc.tensor.transpose(gt_ps[:], gl_sb[:, c * 128:(c + 1) * 128],
                                        ident[0:4, 0:4])
                    rmax = wkp.tile([128, 1], F32, tag="rmax", name="rmax")
                    nc.vector.tensor_reduce(rmax[:], gt_ps[:], mybir.AxisListType.X, ALU.max)
                    nc.vector.tensor_scalar(rmax[:], rmax[:], -1.0, None, ALU.mult)
                    e_sb = wkp.tile([128, 4], F32, tag="e_sb", name="e_sb")
                    nc.scalar.activation(e_sb[:], gt_ps[:], ACTF.Exp, bias=rmax[:])
                    rsum = wkp.tile([128, 1], F32, tag="rsum", name="rsum")
                    nc.vector.tensor_reduce(rsum[:], e_sb[:], mybir.AxisListType.X, ALU.add)
                    nc.vector.reciprocal(rsum[:], rsum[:])
                    nc.vector.tensor_scalar(gw[:, c, :], e_sb[:], rsum[:], None, ALU.mult)

            # ================= stage C: gate apply + proj + MLP =================
            with ExitStack() as ctx:
                wkp = ctx.enter_context(tc.tile_pool(name="wkD", bufs=2))
                sb = ctx.enter_context(tc.tile_pool(name="sb3", bufs=1))
                psm = ctx.enter_context(tc.tile_pool(name="psm3", bufs=1, space="PSUM"))
                gsp = sb.tile([128, KC_E, N], F32R, name="gsp")
                for c in range(KC_E):
                    acc_g = wkp.tile([128, N], F32, tag="acc_g", name="acc_g")
                    for t in range(T):
                        s_rb = wkp.tile([128, N], BF16, tag="s_rb", name="s_rb")
                        nc.sync.dma_start(s_rb[:], s_dram[t, c])
                        w_ap = gw[:, c, t:t + 1]
                        if t == 0:
                            nc.vector.tensor_scalar(acc_g[:], s_rb[:], w_ap, None, ALU.mult)
                        else:
                            nc.vector.scalar_tensor_tensor(acc_g[:], s_rb[:], w_ap,
                                                           acc_g[:], ALU.mult, ALU.add)
                    nc.vector.tensor_scalar(gsp[:, c, :], acc_g[:], THRESH, None, ALU.is_gt)

                search_sb = sb.tile([128, KC_C, N], F32, name="search_sb")
                nc.sync.dma_start(search_sb[:],
                                  search_d[b].rearrange("(k p) n -> p k n", p=128))
                s2 = sb.tile([128, KC_C, N], F32, name="s2")
                y1 = sb.tile([128, KC_C, N], F32R, name="y1")
                pj_pts = [psm.tile([128, 2, 512], F32, tag=f"pw{mc}", name=f"pj_ps{mc}")
                          for mc in range(KC_C)]
                first = True
                for kc in range(KC_E):
                    for hl in ('hi', 'lo'):
                        wtl = load_w(wkp, f'wproj_{hl}', kc, "w_small")
                        for mc in range(KC_C):
                            for ih, (n0, nsz) in enumerate(NHS):
                                nc.tensor.matmul(pj_pts[mc][:, ih, :nsz],
                                                 wtl[:, mc * 128:(mc + 1) * 128],
                                                 gsp[:, kc, n0:n0 + nsz],
                                                 start=first,
                                                 stop=(kc == KC_E - 1 and hl == 'lo'))
                        first = False
                for mc in range(KC_C):
                    pt = pj_pts[mc]
                    nc.vector.scalar_tensor_tensor(s2[:, mc, :], search_sb[:, mc, :],
                                                   1.0, pt[:, :, 0:288], ALU.mult, ALU.add)
                    nc.vector.tensor_scalar(y1[:, mc, :], s2[:, mc, :], THRESH, None,
                                            ALU.is_gt)
                y2 = sb.tile([128, KC_E, N], F32R, name="y2")
                for mcg in range(0, KC_E, 3):
                    f1_pts = [psm.tile([128, 2, 512], F32, tag=f"pw{i}", name=f"f1_ps{i}")
                              for i in range(3)]
                    first = True
                    for kc in range(KC_C):
                        for hl in ('hi', 'lo'):
                            wtl = load_w(wkp, f'wfc1_{hl}', kc, "w_big")
                            for i in range(3):
                                mc = mcg + i
                                for ih, (n0, nsz) in enumerate(NHS):
                                    nc.tensor.matmul(f1_pts[i][:, ih, :nsz],
                                                     wtl[:, mc * 128:(mc + 1) * 128],
                                                     y1[:, kc, n0:n0 + nsz],
                                                     start=first,
                                                     stop=(kc == KC_C - 1 and hl == 'lo'))
                            first = False
                    for i in range(3):
                        nc.vector.tensor_scalar(y2[:, mcg + i, :],
                                                f1_pts[i][:, :, 0:288], THRESH,
                                                None, ALU.is_gt)
                f2_pts = [psm.tile([128, 2, 512], F32, tag=f"pw{mc}", name=f"f2_ps{mc}")
                          for mc in range(KC_C)]
                first = True
                for kc in range(KC_E):
                    for hl in ('hi', 'lo'):
                        wtl = load_w(wkp, f'wfc2_{hl}', kc, "w_small")
                        for mc in range(KC_C):
                            for ih, (n0, nsz) in enumerate(NHS):
                                nc.tensor.matmul(f2_pts[mc][:, ih, :nsz],
                                                 wtl[:, mc * 128:(mc + 1) * 128],
                                                 y2[:, kc, n0:n0 + nsz],
                                                 start=first,
                                                 stop=(kc == KC_E - 1 and hl == 'lo'))
                        first = False
                for mc in range(KC_C):
                    pt = f2_pts[mc]
                    out_sb = wkp.tile([128, N], F32, tag="out_sb", name="out_sb")
                    nc.vector.scalar_tensor_tensor(out_sb[:], s2[:, mc, :], 1.0,
                                                   pt[:, :, 0:288], ALU.mult, ALU.add)
                    nc.sync.dma_start(
                        out_d[b].rearrange("(k p) n -> p k n", p=128)[:, mc, :],
                        out_sb[:])

    nc.compile()
    return nc


def kernel(templates, search, params):
    global _BUILT
    templates = np.asarray(templates, np.float32)
    search = np.asarray(search, np.float32)
    p = {k: np.asarray(v) for k, v in params.items()}
    if _BUILT is None:
        _BUILT = _build()
    nc = _BUILT
    wd = _prep_weights(p)
    tpos = np.asarray(p['t_pos'], np.float32).reshape(T, 1, DIM, N)
    spos = np.asarray(p['s_pos'], np.float32).reshape(1, DIM, N)
    tmpl_pre = templates.reshape(T, 16, DIM, N) + tpos       # [T,16,DIM,N]
    sr_pre = search.reshape(1, 16, DIM, N)[0] + spos         # [16,DIM,N]
    in_maps = []
    for core in range(8):
        m = dict(wd)
        m['templates'] = np.ascontiguousarray(tmpl_pre[:, 2 * core:2 * core + 2])
        m['search_pos'] = np.ascontiguousarray(sr_pre[2 * core:2 * core + 2])
        m['search'] = np.ascontiguousarray(
            search[0, 2 * core:2 * core + 2].reshape(NB, DIM, N))
        m['zeros16'] = np.zeros((16, N), np.float16)
        in_maps.append(m)
    r = run_bass_kernel_spmd(nc, in_maps, core_ids=list(range(8)))
    kernel.last_results = r
    out = np.stack([r.results[c]['out'] for c in range(8)])
    return out.reshape(1, 16, DIM, PATCH, PATCH).astype(np.float32)


# revision 15
# speedup vs baseline: 1.0868x; 1.0868x over previous
# Trainium2 Bass kernel for nn_MemoryRetrieval (spiking memory-retrieval block).
# Data-parallel over batch B=16 across 8 cores (2 batch elements per core).
# All weight matmuls: fp32r hi/lo 2-pass (~fp32-exact); spikes/kv are exact in fp32r.
import sys
sys.path.insert(0, '/opt/trn_rl_repo')
from contextlib import ExitStack

import numpy as np

import concourse.bass as bass
import concourse.tile as tile
import concourse.mybir as mybir
from concourse import bacc
from concourse.bass_utils import run_bass_kernel_spmd
from concourse.masks import make_identity

F32 = mybir.dt.float32
F32R = mybir.dt.float32r
BF16 = mybir.dt.bfloat16
F16 = mybir.dt.float16
ALU = mybir.AluOpType
ACTF = mybir.ActivationFunctionType

DECAY = 0.25
THRESH = 0.5
DIM, HEADS, EXP, PATCH = 384, 8, 1536, 24
N = PATCH * PATCH            # 576
CH = DIM // HEADS            # 48
DH = EXP // HEADS            # 192
SCALE2 = 2.0 * (CH ** -0.5)  # attn scale*2 folded into LIF threshold
THR_ATTN = THRESH / SCALE2
NB = 2
T = 4
KC_C = DIM // 128            # 3
KC_E = EXP // 128            # 12
NCH = [(0, 128), (128, 128), (256, 128), (384, 128), (512, 64)]
NHS = [(0, 288), (288, 288)]

DEBUG_TAPS = False


def _round12(a):
    a = np.asarray(a, np.float32)
    m, e = np.frexp(a)
    scale = np.ldexp(np.float32(1.0), (12 - e.astype(np.int32)))
    with np.errstate(invalid='ignore'):
        r = (np.round(a.astype(np.float64) * scale) / scale).astype(np.float32)
    return np.where(a == 0.0, np.float32(0.0), r)


def _hilo(w):
    hi = _round12(w)
    lo = _round12((w - hi).astype(np.float32))
    return np.ascontiguousarray(hi), np.ascontiguousarray(lo)


def _prep_weights(p):
    d = {}

    def conv_w(name, w, s):
        wT = (w * s[:, None]).T.astype(np.float32)  # [Cin, Cout]
        hi, lo = _hilo(wT)
        d[f'{name}_hi'], d[f'{name}_lo'] = hi, lo

    conv_w('wk', p['wk'], p['sk'])
    conv_w('wv', p['wv'], p['sv'])
    conv_w('wq', p['wq'], p['sq'])
    conv_w('wfuse', p['wfuse'], p['sfuse'])
    conv_w('wproj', p['wproj'], p['sproj'])
    conv_w('wfc1', p['wfc1'], p['sfc1'])
    conv_w('wfc2', p['wfc2'], p['sfc2'])
    for t in range(T):
        conv_w(f'wpw{t}', p['wpw'][t], p['spw'][t])
    hi, lo = _hilo((p['wgate'] / float(N)).astype(np.float32))
    d['wgate_hi'], d['wgate_lo'] = hi, lo
    wdw = (p['wdw'][:, :, 0] * p['sdw'][:, :, None, None]).astype(np.float32)
    d['wdw'] = np.ascontiguousarray(
        wdw.reshape(T, KC_E, 128, 9).transpose(0, 2, 1, 3).reshape(T, 128, KC_E * 9))
    for nm in ('bk', 'bv', 'bq', 'bdw', 'bpw', 'bfuse', 'bproj', 'bfc1', 'bfc2'):
        if np.any(np.asarray(p[nm])):
            raise NotImplementedError(f'nonzero bias {nm} not supported')
    return d


_BUILT = None


def _remap_pad_dmas(nc, dst_pad, src_cmaj):
    """scatter [384ch Cmaj] -> padded-head [128, 4, N] (head h -> tile h//2, rows 64*(h%2))."""
    for h in range(HEADS):
        r0 = h * CH
        dst0 = 64 * (h % 2)
        while r0 < (h + 1) * CH:
            kc, rr = divmod(r0, 128)
            span = min((h + 1) * CH - r0, 128 - rr)
            o = r0 - h * CH
            nc.sync.dma_start(dst_pad[dst0 + o: dst0 + o + span, h // 2, :],
                              src_cmaj[rr:rr + span, kc, :])
            r0 += span


def _build():
    nc = bacc.Bacc("TRN2", target_bir_lowering=False, debug=False)

    templ_d = nc.dram_tensor("templates", [T, NB, DIM, N], F32, kind="ExternalInput").ap()
    search_d = nc.dram_tensor("search", [NB, DIM, N], F32, kind="ExternalInput").ap()
    srpos_d = nc.dram_tensor("search_pos", [NB, DIM, N], F32, kind="ExternalInput").ap()
    w_d = {}
    for nm, cin, cout in (('wk', DIM, DIM), ('wv', DIM, EXP), ('wq', DIM, DIM),
                          ('wfuse', DIM, DIM), ('wproj', EXP, DIM),
                          ('wfc1', DIM, EXP), ('wfc2', EXP, DIM), ('wgate', EXP, EXP),
                          *[(f'wpw{t}', EXP, DIM) for t in range(T)]):
        for hl in ('hi', 'lo'):
            w_d[f'{nm}_{hl}'] = nc.dram_tensor(f'{nm}_{hl}', [cin, cout], F32R,
                                               kind="ExternalInput").ap()
    wdw_d = nc.dram_tensor("wdw", [T, 128, KC_E * 9], F32, kind="ExternalInput").ap()
    zeros_d = nc.dram_tensor("zeros16", [16, N], F16, kind="ExternalInput").ap()
    out_d = nc.dram_tensor("out", [NB, DIM, N], F32, kind="ExternalOutput").ap()

    with tile.TileContext(nc) as tc, ExitStack() as octx:
        st = octx.enter_context(tc.tile_pool(name="carry", bufs=1))
        dr = octx.enter_context(tc.tile_pool(name="dram", bufs=1, space="DRAM"))

        ident = st.tile([128, 128], F32, name="ident")
        make_identity(nc, ident[:])

        def load_w(pool, nm, kc, tag):
            ap = w_d[nm]
            tl = pool.tile([128, ap.shape[1]], F32R, tag=tag, name=f"w_{nm}_{kc}")
            nc.sync.dma_start(tl[:], ap[kc * 128:(kc + 1) * 128, :])
            return tl

        def lif_step(t, mem, g, x_ap, spike, thr=THRESH, accum=None, sdt=None):
            if t == 0:
                nc.vector.tensor_scalar(mem, x_ap, 1.0, None, ALU.mult)
            else:
                nc.vector.scalar_tensor_tensor(g, mem, thr, mem, ALU.is_le, ALU.mult)
                nc.vector.scalar_tensor_tensor(mem, g, DECAY, x_ap, ALU.mult, ALU.add)
            if spike is not None:
                nc.vector.tensor_scalar(spike, mem, thr, None, ALU.is_gt)
                if accum is not None:
                    nc.vector.tensor_reduce(accum, spike, mybir.AxisListType.X,
                                            ALU.add)

        for b in range(NB):
            kv_pad = st.tile([128, T, 4, DH], F16, name="kv_pad")
            sc_cmaj = st.tile([128, KC_C, N], F32, name="sc_cmaj")
            qs_pad = st.tile([128, 4, N], F32, name="qs_pad")
            pooled = st.tile([128, KC_E, T], F32, name="pooled")
            s_dram = dr.tile([T, KC_E, 128, N], BF16, name="s_dram")
            fu_dram = dr.tile([T, KC_C, 128, N], F16, name="fu_dram")
            nc.any.memset(qs_pad[:], 0.0)

            # ================= stage A =================
            with ExitStack() as ctx:
                wkp = ctx.enter_context(tc.tile_pool(name="wkA", bufs=1))
                sa = ctx.enter_context(tc.tile_pool(name="sa", bufs=1))
                # --- sr spike + q conv ---
                with ExitStack() as qctx:
                    psq = qctx.enter_context(tc.tile_pool(name="psq", bufs=2, space="PSUM"))
                    xq = sa.tile([128, KC_C, N], F32, name="xq")
                    nc.sync.dma_start(xq[:], srpos_d[b].rearrange("(k p) n -> p k n", p=128))
                    sr_sp = sa.tile([128, KC_C, N], F32R, name="sr_sp")
                    nc.vector.tensor_scalar(sr_sp[:], xq[:], THRESH, None, ALU.is_gt)
                    for mc in range(KC_C):
                        pt = psq.tile([128, 2, 512], F32, tag="mm", name="q_ps")
                        first = True
                        for kc in range(KC_C):
                            for hl in ('hi', 'lo'):
                                wtl = load_w(wkp, f'wq_{hl}', kc, "w_small")
                                for ih, (n0, nsz) in enumerate(NHS):
                                    nc.tensor.matmul(
                                        pt[:, ih, :nsz], wtl[:, mc * 128:(mc + 1) * 128],
                                        sr_sp[:, kc, n0:n0 + nsz],
                                        start=first, stop=(kc == KC_C - 1 and hl == 'lo'))
                                first = False
                        for ih, (n0, nsz) in enumerate(NHS):
                            nc.any.tensor_copy(sc_cmaj[:, mc, n0:n0 + nsz], pt[:, ih, :nsz])
                _remap_pad_dmas(nc, qs_pad, sc_cmaj)

                # --- per-t: tm LIF, k/v transposed convs + LIF, kv ---
                psa = ctx.enter_context(tc.tile_pool(name="psa", bufs=1, space="PSUM"))
                psk = ctx.enter_context(tc.tile_pool(name="psk", bufs=1, space="PSUM"))
                wres = ctx.enter_context(tc.tile_pool(name="wres", bufs=1))
                wk_res = {}
                wv_res = {}
                for kc in range(KC_C):
                    for hl in ('hi', 'lo'):
                        wk_res[kc, hl] = load_w(wres, f'wk_{hl}', kc, f"wkr{kc}{hl}")
                        wv_res[kc, hl] = load_w(wres, f'wv_{hl}', kc, f"wvr{kc}{hl}")
                mem_tm = sa.tile([128, KC_C, N], F32, name="mem_tm")
                g_tm = sa.tile([128, KC_C, N], F32, name="g_tm")
                mem_k = sa.tile([128, 5, DIM], F32, name="mem_k")
                mem_v = sa.tile([128, 5, EXP], F32, name="mem_v")
                for t in range(T):
                    xt = wkp.tile([128, KC_C, N], F32, name="xt")
                    nc.sync.dma_start(xt[:], templ_d[t, b].rearrange("(k p) n -> p k n", p=128))
                    tm_sp = wkp.tile([128, KC_C, N], F32R, name="tm_sp")
                    lif_step(t, mem_tm[:], g_tm[:], xt[:], tm_sp[:])

                    kT_sp = sa.tile([128, 5, DIM], F16, name="kT_sp")
                    kvp = [psk.tile([128, 384], F32, tag=f"kv{p_}", name=f"kvp{p_}")
                           for p_ in range(4)]
                    for p_ in range(4):
                        nc.vector.memset(kvp[p_][:], 0.0)
                    for inc, (n0, nsz) in enumerate(NCH):
                        ktp = psa.tile([128, DIM], F32, tag="p384", name="ktp")
                        vtp = psa.tile([128, EXP], F32, tag="big", name="vtp")
                        first = True
                        for kc in range(KC_C):
                            for hl in ('hi', 'lo'):
                                wk_tl = wk_res[kc, hl]
                                wv_tl = wv_res[kc, hl]
                                lhsT = tm_sp[:, kc, n0:n0 + nsz]
                                last = (kc == KC_C - 1 and hl == 'lo')
                                nc.tensor.matmul(ktp[:nsz, :], lhsT, wk_tl[:, :],
                                                 start=first, stop=last)
                                for j in range(3):
                                    nc.tensor.matmul(vtp[:nsz, j * 512:(j + 1) * 512],
                                                     lhsT, wv_tl[:, j * 512:(j + 1) * 512],
                                                     start=first, stop=last)
                                first = False
                        g_nm = wkp.tile([128, EXP], F32, tag="g_nm", name="g_nm")
                        vT_sp = wkp.tile([128, EXP], F16, tag="vT_sp", name="vT_sp")
                        lif_step(t, mem_k[:nsz, inc, :], g_nm[:nsz, :DIM],
                                 ktp[:nsz, :], kT_sp[:nsz, inc, :])
                        lif_step(t, mem_v[:nsz, inc, :], g_nm[:nsz, :],
                                 vtp[:nsz, :], vT_sp[:nsz, :])
                        st_ = inc == 0
                        sp_ = inc == len(NCH) - 1
                        for p_ in range(4):
                            nc.tensor.matmul(kvp[p_][0:48, :],
                                             kT_sp[:nsz, inc, 96 * p_:96 * p_ + 48],
                                             vT_sp[:nsz, 384 * p_:384 * p_ + 384],
                                             start=st_, stop=sp_)
                            nc.tensor.matmul(kvp[p_][64:112, :],
                                             kT_sp[:nsz, inc, 96 * p_ + 48:96 * p_ + 96],
                                             vT_sp[:nsz, 384 * p_:384 * p_ + 384],
                                             start=st_, stop=sp_)
                    for p_ in range(4):
                        nc.any.tensor_copy(kv_pad[0:64, t, p_, :], kvp[p_][0:64, 0:DH])
                        nc.any.tensor_copy(kv_pad[64:128, t, p_, :], kvp[p_][64:128, DH:2 * DH])

            def attn_mms(t, c, rhs_pad, pt):
                p_, r = divmod(c, 3)
                for ih, (n0, nsz) in enumerate(NHS):
                    rhs_e = rhs_pad[0:64, p_, n0:n0 + nsz]
                    rhs_o = rhs_pad[64:128, p_, n0:n0 + nsz]
                    if r == 0:
                        nc.tensor.matmul(pt[0:128, ih, :nsz],
                                         kv_pad[0:64, t, p_, 0:128], rhs_e)
                    elif r == 1:
                        nc.tensor.matmul(pt[0:64, ih, :nsz],
                                         kv_pad[0:64, t, p_, 128:192], rhs_e)
                        nc.tensor.matmul(pt[64:128, ih, :nsz],
                                         kv_pad[64:128, t, p_, 0:64], rhs_o,
                                         tile_position=(64, 64))
                    else:
                        nc.tensor.matmul(pt[0:128, ih, :nsz],
                                         kv_pad[64:128, t, p_, 64:192], rhs_o,
                                         tile_position=(64, 0))

            # ================= stage B1: attn1 -> dw -> pw -> fuse =================
            with ExitStack() as ctx:
                wkp = ctx.enter_context(tc.tile_pool(name="wkB", bufs=2))
                sb = ctx.enter_context(tc.tile_pool(name="sb1", bufs=1))
                mem_q = sb.tile([128, 4, N], F32, name="mem_q")
                mem_qq = sb.tile([128, KC_E, N], F32, name="mem_qq")
                mem_dw = sb.tile([128, KC_E, N], F32, name="mem_dw")
                mem_pw = sb.tile([128, KC_C, N], F32, name="mem_pw")
                mem_fu = sb.tile([128, KC_C, N], F32, name="mem_fu")
                dw_sp = sb.tile([128, KC_E, N], F32R, name="dw_sp")
                fu_in = sb.tile([128, KC_C, N], F32R, name="fu_in")
                fu_sp = sb.tile([128, KC_C, N], F16, name="fu_sp")
                q_sp = sb.tile([128, 4, N], F16, name="q_sp")
                wdw_sb = sb.tile([128, T, KC_E * 9], F32, name="wdw_sb")
                nc.sync.dma_start(wdw_sb[:], wdw_d.rearrange("t p k -> p t k"))
                for t in range(T):
                    for p_ in range(4):
                        g_c = wkp.tile([128, N], F32, tag="g_c", name="g_cq")
                        lif_step(t, mem_q[:, p_, :], g_c[:], qs_pad[:, p_, :],
                                 q_sp[:, p_, :])
                    psm = ctx2 = tc.tile_pool(name="psA1", bufs=3, space="PSUM")
                    psm = psm.__enter__()
                    for c in range(KC_E):
                        pt = psm.tile([128, 2, 512], F32, tag="mm", name="a1_ps")
                        attn_mms(t, c, q_sp, pt)
                        g_c = wkp.tile([128, N], F32, tag="g_c", name="g_c1")
                        x_view = pt[:, :, 0:288]
                        if t == 0:
                            nc.vector.tensor_scalar(mem_qq[:, c, :], x_view, 1.0,
                                                    None, ALU.mult)
                        else:
                            nc.vector.scalar_tensor_tensor(g_c[:], mem_qq[:, c, :],
                                                           THR_ATTN, mem_qq[:, c, :],
                                                           ALU.is_le, ALU.mult)
                            nc.vector.scalar_tensor_tensor(mem_qq[:, c, :], g_c[:],
                                                           DECAY, x_view, ALU.mult, ALU.add)
                        dpad = wkp.tile([128, 26, 26], F32, tag="dpad", name="dpad")
                        nc.any.memset(dpad[:], 0.0)
                        nc.vector.tensor_scalar(dpad[:, 1:25, 1:25], mem_qq[:, c, :],
                                                THR_ATTN, None, ALU.is_gt)
                        acc_d = wkp.tile([128, 24, 24], F32, tag="dacc", name="dacc")
                        acc_g = wkp.tile([128, 24, 24], F32, tag="daccg", name="daccg")
                        for tap in range(6):
                            dy, dx = divmod(tap, 3)
                            w_ap = wdw_sb[:, t, c * 9 + tap: c * 9 + tap + 1]
                            view = dpad[:, dy:dy + 24, dx:dx + 24]
                            if tap == 0:
                                nc.vector.tensor_scalar(acc_d[:], view, w_ap, None, ALU.mult)
                            else:
                                nc.vector.scalar_tensor_tensor(acc_d[:], view, w_ap,
                                                               acc_d[:], ALU.mult, ALU.add)
                        for tap in range(6, 9):
                            dy, dx = divmod(tap, 3)
                            w_ap = wdw_sb[:, t, c * 9 + tap: c * 9 + tap + 1]
                            view = dpad[:, dy:dy + 24, dx:dx + 24]
                            if tap == 6:
                                nc.scalar.activation(acc_g[:], view, ACTF.Copy,
                                                     scale=w_ap)
                            else:
                                ztap = wkp.tile([128, 24, 24], F32, tag="ztap",
                                                name="ztap")
                                nc.scalar.activation(ztap[:], view, ACTF.Copy,
                                                     scale=w_ap)
                                nc.gpsimd.tensor_tensor(acc_g[:], acc_g[:], ztap[:],
                                                        ALU.add)
                        # dw LIF with two partial accumulators
                        if t == 0:
                            nc.vector.tensor_scalar(mem_dw[:, c, :],
                                                    acc_d[:].rearrange("p a b -> p (a b)"),
                                                    1.0, None, ALU.mult)
                        else:
                            nc.vector.scalar_tensor_tensor(g_c[:], mem_dw[:, c, :],
                                                           THRESH, mem_dw[:, c, :],
                                                           ALU.is_le, ALU.mult)
                            nc.vector.scalar_tensor_tensor(
                                mem_dw[:, c, :], g_c[:], DECAY,
                                acc_d[:].rearrange("p a b -> p (a b)"),
                                ALU.mult, ALU.add)
                        nc.vector.tensor_tensor(mem_dw[:, c, :], mem_dw[:, c, :],
                                                acc_g[:].rearrange("p a b -> p (a b)"),
                                                ALU.add)
                        nc.vector.tensor_scalar(dw_sp[:, c, :], mem_dw[:, c, :],
                                                THRESH, None, ALU.is_gt)

                    ctx2.__exit__(None, None, None)
                    pspw_cm = tc.tile_pool(name="psB1", bufs=1, space="PSUM")
                    pspw = pspw_cm.__enter__()
                    pw_pts = [pspw.tile([128, 2, 512], F32, tag=f"pw{mc}", name=f"pw_ps{mc}")
                              for mc in range(KC_C)]
                    first = True
                    for kc in range(KC_E):
                        for hl in ('hi', 'lo'):
                            wtl = load_w(wkp, f'wpw{t}_{hl}', kc, "w_small")
                            for mc in range(KC_C):
                                for ih, (n0, nsz) in enumerate(NHS):
                                    nc.tensor.matmul(pw_pts[mc][:, ih, :nsz],
                                                     wtl[:, mc * 128:(mc + 1) * 128],
                                                     dw_sp[:, kc, n0:n0 + nsz],
                                                     start=first,
                                                     stop=(kc == KC_E - 1 and hl == 'lo'))
                            first = False
                    for mc in range(KC_C):
                        pt = pw_pts[mc]
                        g_c = wkp.tile([128, N], F32, tag="g_c", name="g_c2")
                        if t == 0:
                            nc.vector.tensor_scalar(mem_pw[:, mc, :], pt[:, :, 0:288],
                                                    1.0, None, ALU.mult)
                        else:
                            nc.vector.scalar_tensor_tensor(g_c[:], mem_pw[:, mc, :],
                                                           THRESH, mem_pw[:, mc, :],
                                                           ALU.is_le, ALU.mult)
                            nc.vector.scalar_tensor_tensor(mem_pw[:, mc, :], g_c[:],
                                                           DECAY, pt[:, :, 0:288],
                                                           ALU.mult, ALU.add)
                        nc.vector.tensor_tensor(mem_pw[:, mc, :], mem_pw[:, mc, :],
                                                sc_cmaj[:, mc, :], ALU.add)
                        nc.vector.tensor_scalar(fu_in[:, mc, :], mem_pw[:, mc, :],
                                                THRESH, None, ALU.is_gt)

                    fu_pts = [pspw.tile([128, 2, 512], F32, tag=f"pw{mc}", name=f"fu_ps{mc}")
                              for mc in range(KC_C)]
                    first = True
                    for kc in range(KC_C):
                        for hl in ('hi', 'lo'):
                            wtl = load_w(wkp, f'wfuse_{hl}', kc, "w_small")
                            for mc in range(KC_C):
                                for ih, (n0, nsz) in enumerate(NHS):
                                    nc.tensor.matmul(fu_pts[mc][:, ih, :nsz],
                                                     wtl[:, mc * 128:(mc + 1) * 128],
                                                     fu_in[:, kc, n0:n0 + nsz],
                                                     start=first,
                                                     stop=(kc == KC_C - 1 and hl == 'lo'))
                            first = False
                    for mc in range(KC_C):
                        pt = fu_pts[mc]
                        g_c = wkp.tile([128, N], F32, tag="g_c", name="g_c3")
                        lif_step(t, mem_fu[:, mc, :], g_c[:], pt[:, :, 0:288],
                                 fu_sp[:, mc, :])
                        nc.sync.dma_start(fu_dram[t, mc], fu_sp[:, mc, :])
                    pspw_cm.__exit__(None, None, None)

            # ================= stage B2: attn2 + gate =================
            with ExitStack() as ctx:
                wkp = ctx.enter_context(tc.tile_pool(name="wkC", bufs=2))
                sb = ctx.enter_context(tc.tile_pool(name="sb2", bufs=1))
                psm2_cm = tc.tile_pool(name="psm2", bufs=3, space="PSUM")
                psm = psm2_cm.__enter__()
                mem_at = sb.tile([128, KC_E, N], F32, name="mem_at")
                for t in range(T):
                    fu_pad = wkp.tile([128, 4, N], F16, tag="fu_pad", name="fu_pad")
                    for pp in range(4):
                        nc.sync.dma_start(fu_pad[48:64, pp, :], zeros_d)
                        nc.sync.dma_start(fu_pad[112:128, pp, :], zeros_d)
                    for h in range(HEADS):
                        r0 = h * CH
                        dst0 = 64 * (h % 2)
                        while r0 < (h + 1) * CH:
                            kc, rr = divmod(r0, 128)
                            span = min((h + 1) * CH - r0, 128 - rr)
                            o = r0 - h * CH
                            nc.sync.dma_start(
                                fu_pad[dst0 + o:dst0 + o + span, h // 2, :],
                                fu_dram[t, kc, rr:rr + span, :])
                            r0 += span
                    for c in range(KC_E):
                        pt = psm.tile([128, 2, 512], F32, tag="mm", name="a2_ps")
                        attn_mms(t, c, fu_pad, pt)
                        g_c = wkp.tile([128, N], F32, tag="g_c", name="g_c4")
                        s_t = wkp.tile([128, N], BF16, tag="s_t", name="s_t")
                        lif_step(t, mem_at[:, c, :], g_c[:], pt[:, :, 0:288],
                                 s_t[:], thr=THR_ATTN, accum=pooled[:, c, t:t + 1])
                        nc.sync.dma_start(s_dram[t, c], s_t[:])
                psm2_cm.__exit__(None, None, None)
                psg = ctx.enter_context(tc.tile_pool(name="psg", bufs=1, space="PSUM"))
                # gate weights
                pooled_r = sb.tile([128, KC_E, T], F32R, name="pooled_r")
                nc.any.tensor_copy(pooled_r[:], pooled[:])
                gl_ps = psg.tile([4, EXP], F32, tag="gl", name="gl_ps")
                first = True
                for kc in range(KC_E):
                    for hl in ('hi', 'lo'):
                        wtl = load_w(wkp, f'wgate_{hl}', kc, "w_big")
                        for j in range(3):
                            nc.tensor.matmul(gl_ps[:, j * 512:(j + 1) * 512],
                                             pooled_r[:, kc, :],
                                             wtl[:, j * 512:(j + 1) * 512],
                                             start=first, stop=(kc == KC_E - 1 and hl == 'lo'))
                        first = False
                gl_sb = sb.tile([4, EXP], F32, name="gl_sb")
                nc.any.tensor_copy(gl_sb[:], gl_ps[:])
                gw = st.tile([128, KC_E, T], F32, name="gw")
                for c in range(KC_E):
                    gt_ps = psg.tile([128, 4], F32, tag="gt", name="gt_ps")
                    nc.tensor.transpose(gt_ps[:], gl_sb[:, c * 128:(c + 1) * 128],
                                        ident[0:4, 0:4])
                    rmax = wkp.tile([128, 1], F32, tag="rmax", name="rmax")
                    nc.vector.tensor_reduce(rmax[:], gt_ps[:], mybir.AxisListType.X, ALU.max)
                    nc.vector.tensor_scalar(rmax[:], rmax[:], -1.0, None, ALU.mult)
                    e_sb = wkp.tile([128, 4], F32, tag="e_sb", name="e_sb")
                    nc.scalar.activation(e_sb[:], gt_ps[:], ACTF.Exp, bias=rmax[:])
                    rsum = wkp.tile([128, 1], F32, tag="rsum", name="rsum")
                    nc.vector.tensor_reduce(rsum[:], e_sb[:], mybir.AxisListType.X, ALU.add)
                    nc.vector.reciprocal(rsum[:], rsum[:])
                    nc.vector.tensor_scalar(gw[:, c, :], e_sb[:], rsum[:], None, ALU.mult)

            # ================= stage C: gate apply + proj + MLP =================
            with ExitStack() as ctx:
                wkp = ctx.enter_context(tc.tile_pool(name="wkD", bufs=2))
                sb = ctx.enter_context(tc.tile_pool(name="sb3", bufs=1))
                psm = ctx.enter_context(tc.tile_pool(name="psm3", bufs=1, space="PSUM"))
                gsp = sb.tile([128, KC_E, N], F32R, name="gsp")
                for c in range(KC_E):
                    acc_g = wkp.tile([128, N], F32, tag="acc_g", name="acc_g")
                    for t in range(T):
                        s_rb = wkp.tile([128, N], BF16, tag="s_rb", name="s_rb")
                        nc.sync.dma_start(s_rb[:], s_dram[t, c])
                        w_ap = gw[:, c, t:t + 1]
                        if t == 0:
                            nc.vector.tensor_scalar(acc_g[:], s_rb[:], w_ap, None, ALU.mult)
                        else:
                            nc.vector.scalar_tensor_tensor(acc_g[:], s_rb[:], w_ap,
                                                           acc_g[:], ALU.mult, ALU.add)
                    nc.vector.tensor_scalar(gsp[:, c, :], acc_g[:], THRESH, None, ALU.is_gt)

                search_sb = sb.tile([128, KC_C, N], F32, name="search_sb")
                nc.sync.dma_start(search_sb[:],
                                  search_d[b].rearrange("(k p) n -> p k n", p=128))
                s2 = sb.tile([128, KC_C, N], F32, name="s2")
                y1 = sb.tile([128, KC_C, N], F32R, name="y1")
                pj_pts = [psm.tile([128, 2, 512], F32, tag=f"pw{mc}", name=f"pj_ps{mc}")
                          for mc in range(KC_C)]
                first = True
                for kc in range(KC_E):
                    for hl in ('hi', 'lo'):
                        wtl = load_w(wkp, f'wproj_{hl}', kc, "w_small")
                        for mc in range(KC_C):
                            for ih, (n0, nsz) in enumerate(NHS):
                                nc.tensor.matmul(pj_pts[mc][:, ih, :nsz],
                                                 wtl[:, mc * 128:(mc + 1) * 128],
                                                 gsp[:, kc, n0:n0 + nsz],
                                                 start=first,
                                                 stop=(kc == KC_E - 1 and hl == 'lo'))
                        first = False
                for mc in range(KC_C):
                    pt = pj_pts[mc]
                    nc.vector.scalar_tensor_tensor(s2[:, mc, :], search_sb[:, mc, :],
                                                   1.0, pt[:, :, 0:288], ALU.mult, ALU.add)
                    nc.vector.tensor_scalar(y1[:, mc, :], s2[:, mc, :], THRESH, None,
                                            ALU.is_gt)
                y2 = sb.tile([128, KC_E, N], F32R, name="y2")
                for mcg in range(0, KC_E, 3):
                    f1_pts = [psm.tile([128, 2, 512], F32, tag=f"pw{i}", name=f"f1_ps{i}")
                              for i in range(3)]
                    first = True
                    for kc in range(KC_C):
                        for hl in ('hi', 'lo'):
                            wtl = load_w(wkp, f'wfc1_{hl}', kc, "w_big")
                            for i in range(3):
                                mc = mcg + i
                                for ih, (n0, nsz) in enumerate(NHS):
                                    nc.tensor.matmul(f1_pts[i][:, ih, :nsz],
                                                     wtl[:, mc * 128:(mc + 1) * 128],
                                                     y1[:, kc, n0:n0 + nsz],
                                                     start=first,
                                                     stop=(kc == KC_C - 1 and hl == 'lo'))
                            first = False
                    for i in range(3):
                        nc.vector.tensor_scalar(y2[:, mcg + i, :],
                                                f1_pts[i][:, :, 0:288], THRESH,
                                                None, ALU.is_gt)
                f2_pts = [psm.tile([128, 2, 512], F32, tag=f"pw{mc}", name=f"f2_ps{mc}")
                          for mc in range(KC_C)]
                first = True
                for kc in range(KC_E):
                    for hl in ('hi', 'lo'):
                        wtl = load_w(wkp, f'wfc2_{hl}', kc, "w_small")
                        for mc in range(KC_C):
                            for ih, (n0, nsz) in enumerate(NHS):
                                nc.tensor.matmul(f2_pts[mc][:, ih, :nsz],
                                                 wtl[:, mc * 128:(mc + 1) * 128],
                                                 y2[:, kc, n0:n0 + nsz],
                                                 start=first,
                                                 stop=(kc == KC_E - 1 and hl == 'lo'))
                        first = False
                for mc in range(KC_C):
                    pt = f2_pts[mc]
                    out_sb = wkp.tile([128, N], F32, tag="out_sb", name="out_sb")
                    nc.vector.scalar_tensor_tensor(out_sb[:], s2[:, mc, :], 1.0,
                                                   pt[:, :, 0:288], ALU.mult, ALU.add)
                    nc.sync.dma_start(
                        out_d[b].rearrange("(k p) n -> p k n", p=128)[:, mc, :],
                        out_sb[:])

    nc.compile()
    return nc


def kernel(templates, search, params):
    global _BUILT
    templates = np.asarray(templates, np.float32)
    search = np.asarray(search, np.float32)
    p = {k: np.asarray(v) for k, v in params.items()}
    if _BUILT is None:
        _BUILT = _build()
    nc = _BUILT
    wd = _prep_weights(p)
    tpos = np.asarray(p['t_pos'], np.float32).reshape(T, 1, DIM, N)
    spos = np.asarray(p['s_pos'], np.float32).reshape(1, DIM, N)
    tmpl_pre = templates.reshape(T, 16, DIM, N) + tpos       # [T,16,DIM,N]
    sr_pre = search.reshape(1, 16, DIM, N)[0] + spos         # [16,DIM,N]
    in_maps = []
    for core in range(8):
        m = dict(wd)
        m['templates'] = np.ascontiguousarray(tmpl_pre[:, 2 * core:2 * core + 2])
        m['search_pos'] = np.ascontiguousarray(sr_pre[2 * core:2 * core + 2])
        m['search'] = np.ascontiguousarray(
            search[0, 2 * core:2 * core + 2].reshape(NB, DIM, N))
        m['zeros16'] = np.zeros((16, N), np.float16)
        in_maps.append(m)
    r = run_bass_kernel_spmd(nc, in_maps, core_ids=list(range(8)))
    kernel.last_results = r
    out = np.stack([r.results[c]['out'] for c in range(8)])
    return out.reshape(1, 16, DIM, PATCH, PATCH).astype(np.float32)


# revision 16
# speedup vs baseline: 1.2899x; 1.1869x over previous
# Trainium2 Bass kernel for nn_MemoryRetrieval (spiking memory-retrieval block).
# Data-parallel over batch B=16 across 8 cores (2 batch elements per core).
# All weight matmuls: fp32r hi/lo 2-pass (~fp32-exact); spikes/kv are exact in fp32r.
import sys
sys.path.insert(0, '/opt/trn_rl_repo')
from contextlib import ExitStack

import numpy as np

import concourse.bass as bass
import concourse.tile as tile
import concourse.mybir as mybir
from concourse import bacc
from concourse.bass_utils import run_bass_kernel_spmd
from concourse.masks import make_identity

F32 = mybir.dt.float32
F32R = mybir.dt.float32r
BF16 = mybir.dt.bfloat16
F16 = mybir.dt.float16
ALU = mybir.AluOpType
ACTF = mybir.ActivationFunctionType

DECAY = 0.25
THRESH = 0.5
DIM, HEADS, EXP, PATCH = 384, 8, 1536, 24
N = PATCH * PATCH            # 576
CH = DIM // HEADS            # 48
DH = EXP // HEADS            # 192
SCALE2 = 2.0 * (CH ** -0.5)  # attn scale*2 folded into LIF threshold
THR_ATTN = THRESH / SCALE2
NB = 2
T = 4
KC_C = DIM // 128            # 3
KC_E = EXP // 128            # 12
NCH = [(0, 128), (128, 128), (256, 128), (384, 128), (512, 64)]
NHS = [(0, 288), (288, 288)]

DEBUG_TAPS = False


def _round12(a):
    a = np.asarray(a, np.float32)
    m, e = np.frexp(a)
    scale = np.ldexp(np.float32(1.0), (12 - e.astype(np.int32)))
    with np.errstate(invalid='ignore'):
        r = (np.round(a.astype(np.float64) * scale) / scale).astype(np.float32)
    return np.where(a == 0.0, np.float32(0.0), r)


def _hilo(w):
    hi = _round12(w)
    lo = _round12((w - hi).astype(np.float32))
    return np.ascontiguousarray(hi), np.ascontiguousarray(lo)


def _prep_weights(p):
    d = {}

    def conv_w(name, w, s):
        wT = (w * s[:, None]).T.astype(np.float32)  # [Cin, Cout]
        hi, lo = _hilo(wT)
        d[f'{name}_hi'], d[f'{name}_lo'] = hi, lo

    conv_w('wk', p['wk'], p['sk'])
    conv_w('wv', p['wv'], p['sv'])
    conv_w('wq', p['wq'], p['sq'])
    conv_w('wfuse', p['wfuse'], p['sfuse'])
    conv_w('wproj', p['wproj'], p['sproj'])
    conv_w('wfc1', p['wfc1'], p['sfc1'])
    conv_w('wfc2', p['wfc2'], p['sfc2'])
    for t in range(T):
        conv_w(f'wpw{t}', p['wpw'][t], p['spw'][t])
    hi, lo = _hilo((p['wgate'] / float(N)).astype(np.float32))
    d['wgate_hi'], d['wgate_lo'] = hi, lo
    wdw = (p['wdw'][:, :, 0] * p['sdw'][:, :, None, None]).astype(np.float32)
    d['wdw'] = np.ascontiguousarray(
        wdw.reshape(T, KC_E, 128, 9).transpose(0, 2, 1, 3).reshape(T, 128, KC_E * 9))
    for nm in ('bk', 'bv', 'bq', 'bdw', 'bpw', 'bfuse', 'bproj', 'bfc1', 'bfc2'):
        if np.any(np.asarray(p[nm])):
            raise NotImplementedError(f'nonzero bias {nm} not supported')
    return d


_BUILT = None


def _remap_pad_dmas(nc, dst_pad, src_cmaj):
    """scatter [384ch Cmaj] -> padded-head [128, 4, N] (head h -> tile h//2, rows 64*(h%2))."""
    for h in range(HEADS):
        r0 = h * CH
        dst0 = 64 * (h % 2)
        while r0 < (h + 1) * CH:
            kc, rr = divmod(r0, 128)
            span = min((h + 1) * CH - r0, 128 - rr)
            o = r0 - h * CH
            nc.sync.dma_start(dst_pad[dst0 + o: dst0 + o + span, h // 2, :],
                              src_cmaj[rr:rr + span, kc, :])
            r0 += span


def _build():
    nc = bacc.Bacc("TRN2", target_bir_lowering=False, debug=False)

    templ_d = nc.dram_tensor("templates", [T, NB, DIM, N], F32, kind="ExternalInput").ap()
    search_d = nc.dram_tensor("search", [NB, DIM, N], F32, kind="ExternalInput").ap()
    srpos_d = nc.dram_tensor("search_pos", [NB, DIM, N], F32, kind="ExternalInput").ap()
    w_d = {}
    for nm, cin, cout in (('wk', DIM, DIM), ('wv', DIM, EXP), ('wq', DIM, DIM),
                          ('wfuse', DIM, DIM), ('wproj', EXP, DIM),
                          ('wfc1', DIM, EXP), ('wfc2', EXP, DIM), ('wgate', EXP, EXP),
                          *[(f'wpw{t}', EXP, DIM) for t in range(T)]):
        for hl in ('hi', 'lo'):
            w_d[f'{nm}_{hl}'] = nc.dram_tensor(f'{nm}_{hl}', [cin, cout], F32R,
                                               kind="ExternalInput").ap()
    wdw_d = nc.dram_tensor("wdw", [T, 128, KC_E * 9], F32, kind="ExternalInput").ap()
    zeros_d = nc.dram_tensor("zeros16", [16, N], F16, kind="ExternalInput").ap()
    out_d = nc.dram_tensor("out", [NB, DIM, N], F32, kind="ExternalOutput").ap()

    with tile.TileContext(nc) as tc, ExitStack() as octx:
        st = octx.enter_context(tc.tile_pool(name="carry", bufs=1))
        dr = octx.enter_context(tc.tile_pool(name="dram", bufs=1, space="DRAM"))

        ident = st.tile([128, 128], F32, name="ident")
        make_identity(nc, ident[:])

        def load_w(pool, nm, kc, tag):
            ap = w_d[nm]
            tl = pool.tile([128, ap.shape[1]], F32R, tag=tag, name=f"w_{nm}_{kc}")
            nc.sync.dma_start(tl[:], ap[kc * 128:(kc + 1) * 128, :])
            return tl

        def lif_step(t, mem, g, x_ap, spike, thr=THRESH, accum=None, sdt=None):
            if t == 0:
                nc.vector.tensor_scalar(mem, x_ap, 1.0, None, ALU.mult)
            else:
                nc.vector.scalar_tensor_tensor(g, mem, thr, mem, ALU.is_le, ALU.mult)
                nc.vector.scalar_tensor_tensor(mem, g, DECAY, x_ap, ALU.mult, ALU.add)
            if spike is not None:
                nc.vector.tensor_scalar(spike, mem, thr, None, ALU.is_gt)
                if accum is not None:
                    nc.vector.tensor_reduce(accum, spike, mybir.AxisListType.X,
                                            ALU.add)

        for b in range(NB):
            kv_pad = st.tile([128, T, 4, DH], F16, name="kv_pad")
            sc_cmaj = st.tile([128, KC_C, N], F32, name="sc_cmaj")
            qs_pad = st.tile([128, 4, N], F32, name="qs_pad")
            pooled = st.tile([128, KC_E, T], F32, name="pooled")
            s_dram = dr.tile([T, KC_E, 128, N], BF16, name="s_dram")
            fu_dram = dr.tile([T, KC_C, 128, N], F16, name="fu_dram")
            nc.any.memset(qs_pad[:], 0.0)

            # ================= stage A =================
            with ExitStack() as ctx:
                wkp = ctx.enter_context(tc.tile_pool(name="wkA", bufs=1))
                sa = ctx.enter_context(tc.tile_pool(name="sa", bufs=1))
                # --- sr spike + q conv ---
                with ExitStack() as qctx:
                    psq = qctx.enter_context(tc.tile_pool(name="psq", bufs=2, space="PSUM"))
                    xq = sa.tile([128, KC_C, N], F32, name="xq")
                    nc.sync.dma_start(xq[:], srpos_d[b].rearrange("(k p) n -> p k n", p=128))
                    sr_sp = sa.tile([128, KC_C, N], F32R, name="sr_sp")
                    nc.vector.tensor_scalar(sr_sp[:], xq[:], THRESH, None, ALU.is_gt)
                    for mc in range(KC_C):
                        pt = psq.tile([128, 2, 512], F32, tag="mm", name="q_ps")
                        first = True
                        for kc in range(KC_C):
                            for hl in ('hi', 'lo'):
                                wtl = load_w(wkp, f'wq_{hl}', kc, "w_small")
                                for ih, (n0, nsz) in enumerate(NHS):
                                    nc.tensor.matmul(
                                        pt[:, ih, :nsz], wtl[:, mc * 128:(mc + 1) * 128],
                                        sr_sp[:, kc, n0:n0 + nsz],
                                        start=first, stop=(kc == KC_C - 1 and hl == 'lo'))
                                first = False
                        for ih, (n0, nsz) in enumerate(NHS):
                            nc.any.tensor_copy(sc_cmaj[:, mc, n0:n0 + nsz], pt[:, ih, :nsz])
                _remap_pad_dmas(nc, qs_pad, sc_cmaj)

                # --- per-t: tm LIF, k/v transposed convs + LIF, kv ---
                psa = ctx.enter_context(tc.tile_pool(name="psa", bufs=1, space="PSUM"))
                psk = ctx.enter_context(tc.tile_pool(name="psk", bufs=1, space="PSUM"))
                wres = ctx.enter_context(tc.tile_pool(name="wres", bufs=1))
                wk_res = {}
                wv_res = {}
                for kc in range(KC_C):
                    for hl in ('hi', 'lo'):
                        wk_res[kc, hl] = load_w(wres, f'wk_{hl}', kc, f"wkr{kc}{hl}")
                        wv_res[kc, hl] = load_w(wres, f'wv_{hl}', kc, f"wvr{kc}{hl}")
                mem_tm = sa.tile([128, KC_C, N], F32, name="mem_tm")
                g_tm = sa.tile([128, KC_C, N], F32, name="g_tm")
                mem_k = sa.tile([128, 5, DIM], F32, name="mem_k")
                mem_v = sa.tile([128, 5, EXP], F32, name="mem_v")
                for t in range(T):
                    xt = wkp.tile([128, KC_C, N], F32, name="xt")
                    nc.sync.dma_start(xt[:], templ_d[t, b].rearrange("(k p) n -> p k n", p=128))
                    tm_sp = wkp.tile([128, KC_C, N], F32R, name="tm_sp")
                    lif_step(t, mem_tm[:], g_tm[:], xt[:], tm_sp[:])

                    kT_sp = sa.tile([128, 5, DIM], F16, name="kT_sp")
                    kvp = [psk.tile([128, 384], F32, tag=f"kv{p_}", name=f"kvp{p_}")
                           for p_ in range(4)]
                    for p_ in range(4):
                        nc.vector.memset(kvp[p_][:], 0.0)
                    for inc, (n0, nsz) in enumerate(NCH):
                        ktp = psa.tile([128, DIM], F32, tag="p384", name="ktp")
                        vtp = psa.tile([128, EXP], F32, tag="big", name="vtp")
                        first = True
                        for kc in range(KC_C):
                            for hl in ('hi', 'lo'):
                                wk_tl = wk_res[kc, hl]
                                wv_tl = wv_res[kc, hl]
                                lhsT = tm_sp[:, kc, n0:n0 + nsz]
                                last = (kc == KC_C - 1 and hl == 'lo')
                                nc.tensor.matmul(ktp[:nsz, :], lhsT, wk_tl[:, :],
                                                 start=first, stop=last)
                                for j in range(3):
                                    nc.tensor.matmul(vtp[:nsz, j * 512:(j + 1) * 512],
                                                     lhsT, wv_tl[:, j * 512:(j + 1) * 512],
                                                     start=first, stop=last)
                                first = False
                        g_nm = wkp.tile([128, EXP], F32, tag="g_nm", name="g_nm")
                        vT_sp = wkp.tile([128, EXP], F16, tag="vT_sp", name="vT_sp")
                        lif_step(t, mem_k[:nsz, inc, :], g_nm[:nsz, :DIM],
                                 ktp[:nsz, :], kT_sp[:nsz, inc, :])
                        lif_step(t, mem_v[:nsz, inc, :], g_nm[:nsz, :],
                                 vtp[:nsz, :], vT_sp[:nsz, :])
                        st_ = inc == 0
                        sp_ = inc == len(NCH) - 1
                        for p_ in range(4):
                            nc.tensor.matmul(kvp[p_][0:48, :],
                                             kT_sp[:nsz, inc, 96 * p_:96 * p_ + 48],
                                             vT_sp[:nsz, 384 * p_:384 * p_ + 384],
                                             start=st_, stop=sp_)
                            nc.tensor.matmul(kvp[p_][64:112, :],
                                             kT_sp[:nsz, inc, 96 * p_ + 48:96 * p_ + 96],
                                             vT_sp[:nsz, 384 * p_:384 * p_ + 384],
                                             start=st_, stop=sp_)
                    for p_ in range(4):
                        nc.any.tensor_copy(kv_pad[0:64, t, p_, :], kvp[p_][0:64, 0:DH])
                        nc.any.tensor_copy(kv_pad[64:128, t, p_, :], kvp[p_][64:128, DH:2 * DH])

            def attn_mms(t, c, rhs_pad, pt):
                p_, r = divmod(c, 3)
                for ih, (n0, nsz) in enumerate(NHS):
                    rhs_e = rhs_pad[0:64, p_, n0:n0 + nsz]
                    rhs_o = rhs_pad[64:128, p_, n0:n0 + nsz]
                    if r == 0:
                        nc.tensor.matmul(pt[0:128, ih, :nsz],
                                         kv_pad[0:64, t, p_, 0:128], rhs_e)
                    elif r == 1:
                        nc.tensor.matmul(pt[0:64, ih, :nsz],
                                         kv_pad[0:64, t, p_, 128:192], rhs_e)
                        nc.tensor.matmul(pt[64:128, ih, :nsz],
                                         kv_pad[64:128, t, p_, 0:64], rhs_o,
                                         tile_position=(64, 64))
                    else:
                        nc.tensor.matmul(pt[0:128, ih, :nsz],
                                         kv_pad[64:128, t, p_, 64:192], rhs_o,
                                         tile_position=(64, 0))

            # ================= stage B1: attn1 -> dw -> pw -> fuse =================
            with ExitStack() as ctx:
                wkp = ctx.enter_context(tc.tile_pool(name="wkB", bufs=3))
                sb = ctx.enter_context(tc.tile_pool(name="sb1", bufs=1))
                mem_q = sb.tile([128, 4, N], F32, name="mem_q")
                mem_qq = sb.tile([128, KC_E, N], F32, name="mem_qq")
                mem_dw = sb.tile([128, KC_E, N], F32, name="mem_dw")
                mem_pw = sb.tile([128, KC_C, N], F32, name="mem_pw")
                mem_fu = sb.tile([128, KC_C, N], F32, name="mem_fu")
                dw_sp = sb.tile([128, KC_E, N], F32R, name="dw_sp")
                fu_in = sb.tile([128, KC_C, N], F32R, name="fu_in")
                fu_sp = sb.tile([128, KC_C, N], F16, name="fu_sp")
                q_sp = sb.tile([128, 4, N], F16, name="q_sp")
                wdw_sb = sb.tile([128, T, KC_E * 9], F32, name="wdw_sb")
                nc.sync.dma_start(wdw_sb[:], wdw_d.rearrange("t p k -> p t k"))
                for t in range(T):
                    for p_ in range(4):
                        g_c = wkp.tile([128, N], F32, tag="g_c", name="g_cq")
                        lif_step(t, mem_q[:, p_, :], g_c[:], qs_pad[:, p_, :],
                                 q_sp[:, p_, :])
                    psm = ctx2 = tc.tile_pool(name="psA1", bufs=3, space="PSUM")
                    psm = psm.__enter__()
                    for c in range(KC_E):
                        pt = psm.tile([128, 2, 512], F32, tag="mm", name="a1_ps")
                        attn_mms(t, c, q_sp, pt)
                        g_c = wkp.tile([128, N], F32, tag="g_c", name="g_c1")
                        x_view = pt[:, :, 0:288]
                        if t == 0:
                            nc.vector.tensor_scalar(mem_qq[:, c, :], x_view, 1.0,
                                                    None, ALU.mult)
                        else:
                            nc.vector.scalar_tensor_tensor(g_c[:], mem_qq[:, c, :],
                                                           THR_ATTN, mem_qq[:, c, :],
                                                           ALU.is_le, ALU.mult)
                            nc.vector.scalar_tensor_tensor(mem_qq[:, c, :], g_c[:],
                                                           DECAY, x_view, ALU.mult, ALU.add)
                        dpad = wkp.tile([128, 26, 26], F32, tag="dpad", name="dpad")
                        nc.any.memset(dpad[:], 0.0)
                        nc.vector.tensor_scalar(dpad[:, 1:25, 1:25], mem_qq[:, c, :],
                                                THR_ATTN, None, ALU.is_gt)
                        acc_d = wkp.tile([128, 24, 24], F32, tag="dacc", name="dacc")
                        acc_g = wkp.tile([128, 24, 24], F32, tag="daccg", name="daccg")
                        for tap in range(6):
                            dy, dx = divmod(tap, 3)
                            w_ap = wdw_sb[:, t, c * 9 + tap: c * 9 + tap + 1]
                            view = dpad[:, dy:dy + 24, dx:dx + 24]
                            if tap == 0:
                                nc.vector.tensor_scalar(acc_d[:], view, w_ap, None, ALU.mult)
                            else:
                                nc.vector.scalar_tensor_tensor(acc_d[:], view, w_ap,
                                                               acc_d[:], ALU.mult, ALU.add)
                        for tap in range(6, 9):
                            dy, dx = divmod(tap, 3)
                            w_ap = wdw_sb[:, t, c * 9 + tap: c * 9 + tap + 1]
                            view = dpad[:, dy:dy + 24, dx:dx + 24]
                            if tap == 6:
                                nc.scalar.activation(acc_g[:], view, ACTF.Copy,
                                                     scale=w_ap)
                            else:
                                ztap = wkp.tile([128, 24, 24], F32, tag="ztap",
                                                name="ztap")
                                nc.scalar.activation(ztap[:], view, ACTF.Copy,
                                                     scale=w_ap)
                                nc.gpsimd.tensor_tensor(acc_g[:], acc_g[:], ztap[:],
                                                        ALU.add)
                        # dw LIF with two partial accumulators
                        if t == 0:
                            nc.vector.tensor_scalar(mem_dw[:, c, :],
                                                    acc_d[:].rearrange("p a b -> p (a b)"),
                                                    1.0, None, ALU.mult)
                        else:
                            nc.vector.scalar_tensor_tensor(g_c[:], mem_dw[:, c, :],
                                                           THRESH, mem_dw[:, c, :],
                                                           ALU.is_le, ALU.mult)
                            nc.vector.scalar_tensor_tensor(
                                mem_dw[:, c, :], g_c[:], DECAY,
                                acc_d[:].rearrange("p a b -> p (a b)"),
                                ALU.mult, ALU.add)
                        nc.vector.tensor_tensor(mem_dw[:, c, :], mem_dw[:, c, :],
                                                acc_g[:].rearrange("p a b -> p (a b)"),
                                                ALU.add)
                        nc.vector.tensor_scalar(dw_sp[:, c, :], mem_dw[:, c, :],
                                                THRESH, None, ALU.is_gt)

                    ctx2.__exit__(None, None, None)
                    pspw_cm = tc.tile_pool(name="psB1", bufs=1, space="PSUM")
                    pspw = pspw_cm.__enter__()
                    pw_pts = [pspw.tile([128, 2, 512], F32, tag=f"pw{mc}", name=f"pw_ps{mc}")
                              for mc in range(KC_C)]
                    first = True
                    for kc in range(KC_E):
                        for hl in ('hi', 'lo'):
                            wtl = load_w(wkp, f'wpw{t}_{hl}', kc, "w_small")
                            for mc in range(KC_C):
                                for ih, (n0, nsz) in enumerate(NHS):
                                    nc.tensor.matmul(pw_pts[mc][:, ih, :nsz],
                                                     wtl[:, mc * 128:(mc + 1) * 128],
                                                     dw_sp[:, kc, n0:n0 + nsz],
                                                     start=first,
                                                     stop=(kc == KC_E - 1 and hl == 'lo'))
                            first = False
                    for mc in range(KC_C):
                        pt = pw_pts[mc]
                        g_c = wkp.tile([128, N], F32, tag="g_c", name="g_c2")
                        if t == 0:
                            nc.vector.tensor_scalar(mem_pw[:, mc, :], pt[:, :, 0:288],
                                                    1.0, None, ALU.mult)
                        else:
                            nc.vector.scalar_tensor_tensor(g_c[:], mem_pw[:, mc, :],
                                                           THRESH, mem_pw[:, mc, :],
                                                           ALU.is_le, ALU.mult)
                            nc.vector.scalar_tensor_tensor(mem_pw[:, mc, :], g_c[:],
                                                           DECAY, pt[:, :, 0:288],
                                                           ALU.mult, ALU.add)
                        nc.vector.tensor_tensor(mem_pw[:, mc, :], mem_pw[:, mc, :],
                                                sc_cmaj[:, mc, :], ALU.add)
                        nc.vector.tensor_scalar(fu_in[:, mc, :], mem_pw[:, mc, :],
                                                THRESH, None, ALU.is_gt)

                    fu_pts = [pspw.tile([128, 2, 512], F32, tag=f"pw{mc}", name=f"fu_ps{mc}")
                              for mc in range(KC_C)]
                    first = True
                    for kc in range(KC_C):
                        for hl in ('hi', 'lo'):
                            wtl = load_w(wkp, f'wfuse_{hl}', kc, "w_small")
                            for mc in range(KC_C):
                                for ih, (n0, nsz) in enumerate(NHS):
                                    nc.tensor.matmul(fu_pts[mc][:, ih, :nsz],
                                                     wtl[:, mc * 128:(mc + 1) * 128],
                                                     fu_in[:, kc, n0:n0 + nsz],
                                                     start=first,
                                                     stop=(kc == KC_C - 1 and hl == 'lo'))
                            first = False
                    for mc in range(KC_C):
                        pt = fu_pts[mc]
                        g_c = wkp.tile([128, N], F32, tag="g_c", name="g_c3")
                        lif_step(t, mem_fu[:, mc, :], g_c[:], pt[:, :, 0:288],
                                 fu_sp[:, mc, :])
                        nc.sync.dma_start(fu_dram[t, mc], fu_sp[:, mc, :])
                    pspw_cm.__exit__(None, None, None)

            # ================= stage B2: attn2 + gate =================
            with ExitStack() as ctx:
                wkp = ctx.enter_context(tc.tile_pool(name="wkC", bufs=3))
                sb = ctx.enter_context(tc.tile_pool(name="sb2", bufs=1))
                psm2_cm = tc.tile_pool(name="psm2", bufs=3, space="PSUM")
                psm = psm2_cm.__enter__()
                mem_at = sb.tile([128, KC_E, N], F32, name="mem_at")
                for t in range(T):
                    fu_pad = wkp.tile([128, 4, N], F16, tag="fu_pad", name="fu_pad")
                    for pp in range(4):
                        nc.sync.dma_start(fu_pad[48:64, pp, :], zeros_d)
                        nc.sync.dma_start(fu_pad[112:128, pp, :], zeros_d)
                    for h in range(HEADS):
                        r0 = h * CH
                        dst0 = 64 * (h % 2)
                        while r0 < (h + 1) * CH:
                            kc, rr = divmod(r0, 128)
                            span = min((h + 1) * CH - r0, 128 - rr)
                            o = r0 - h * CH
                            nc.sync.dma_start(
                                fu_pad[dst0 + o:dst0 + o + span, h // 2, :],
                                fu_dram[t, kc, rr:rr + span, :])
                            r0 += span
                    for c in range(KC_E):
                        pt = psm.tile([128, 2, 512], F32, tag="mm", name="a2_ps")
                        attn_mms(t, c, fu_pad, pt)
                        g_c = wkp.tile([128, N], F32, tag="g_c", name="g_c4")
                        s_t = wkp.tile([128, N], BF16, tag="s_t", name="s_t")
                        lif_step(t, mem_at[:, c, :], g_c[:], pt[:, :, 0:288],
                                 s_t[:], thr=THR_ATTN, accum=pooled[:, c, t:t + 1])
                        nc.sync.dma_start(s_dram[t, c], s_t[:])
                psm2_cm.__exit__(None, None, None)
                psg = ctx.enter_context(tc.tile_pool(name="psg", bufs=1, space="PSUM"))
                # gate weights
                pooled_r = sb.tile([128, KC_E, T], F32R, name="pooled_r")
                nc.any.tensor_copy(pooled_r[:], pooled[:])
                gl_ps = psg.tile([4, EXP], F32, tag="gl", name="gl_ps")
                first = True
                for kc in range(KC_E):
                    for hl in ('hi', 'lo'):
                        wtl = load_w(wkp, f'wgate_{hl}', kc, "w_big")
                        for j in range(3):
                            nc.tensor.matmul(gl_ps[:, j * 512:(j + 1) * 512],
                                             pooled_r[:, kc, :],
                                             wtl[:, j * 512:(j + 1) * 512],
                                             start=first, stop=(kc == KC_E - 1 and hl == 'lo'))
                        first = False
                gl_sb = sb.tile([4, EXP], F32, name="gl_sb")
                nc.any.tensor_copy(gl_sb[:], gl_ps[:])
                gw = st.tile([128, KC_E, T], F32, name="gw")
                for c in range(KC_E):
                    gt_ps = psg.tile([128, 4], F32, tag="gt", name="gt_ps")
                    nc.tensor.transpose(gt_ps[:], gl_sb[:, c * 128:(c + 1) * 128],
                                        ident[0:4, 0:4])
                    rmax = wkp.tile([128, 1], F32, tag="rmax", name="rmax")
                    nc.vector.tensor_reduce(rmax[:], gt_ps[:], mybir.AxisListType.X, ALU.max)
                    nc.vector.tensor_scalar(rmax[:], rmax[:], -1.0, None, ALU.mult)
                    e_sb = wkp.tile([128, 4], F32, tag="e_sb", name="e_sb")
                    nc.scalar.activation(e_sb[:], gt_ps[:], ACTF.Exp, bias=rmax[:])
                    rsum = wkp.tile([128, 1], F32, tag="rsum", name="rsum")
                    nc.vector.tensor_reduce(rsum[:], e_sb[:], mybir.AxisListType.X, ALU.add)
                    nc.vector.reciprocal(rsum[:], rsum[:])
                    nc.vector.tensor_scalar(gw[:, c, :], e_sb[:], rsum[:], None, ALU.mult)

            # ================= stage C: gate apply + proj + MLP =================
            with ExitStack() as ctx:
                wkp = ctx.enter_context(tc.tile_pool(name="wkD", bufs=3))
                sb = ctx.enter_context(tc.tile_pool(name="sb3", bufs=1))
                psm = ctx.enter_context(tc.tile_pool(name="psm3", bufs=1, space="PSUM"))
                gsp = sb.tile([128, KC_E, N], F32R, name="gsp")
                for c in range(KC_E):
                    acc_g = wkp.tile([128, N], F32, tag="acc_g", name="acc_g")
                    for t in range(T):
                        s_rb = wkp.tile([128, N], BF16, tag="s_rb", name="s_rb")
                        nc.sync.dma_start(s_rb[:], s_dram[t, c])
                        w_ap = gw[:, c, t:t + 1]
                        if t == 0:
                            nc.vector.tensor_scalar(acc_g[:], s_rb[:], w_ap, None, ALU.mult)
                        else:
                            nc.vector.scalar_tensor_tensor(acc_g[:], s_rb[:], w_ap,
                                                           acc_g[:], ALU.mult, ALU.add)
                    nc.vector.tensor_scalar(gsp[:, c, :], acc_g[:], THRESH, None, ALU.is_gt)

                search_sb = sb.tile([128, KC_C, N], F32, name="search_sb")
                nc.sync.dma_start(search_sb[:],
                                  search_d[b].rearrange("(k p) n -> p k n", p=128))
                s2 = sb.tile([128, KC_C, N], F32, name="s2")
                y1 = sb.tile([128, KC_C, N], F32R, name="y1")
                pj_pts = [psm.tile([128, 2, 512], F32, tag=f"pw{mc}", name=f"pj_ps{mc}")
                          for mc in range(KC_C)]
                first = True
                for kc in range(KC_E):
                    for hl in ('hi', 'lo'):
                        wtl = load_w(wkp, f'wproj_{hl}', kc, "w_small")
                        for mc in range(KC_C):
                            for ih, (n0, nsz) in enumerate(NHS):
                                nc.tensor.matmul(pj_pts[mc][:, ih, :nsz],
                                                 wtl[:, mc * 128:(mc + 1) * 128],
                                                 gsp[:, kc, n0:n0 + nsz],
                                                 start=first,
                                                 stop=(kc == KC_E - 1 and hl == 'lo'))
                        first = False
                for mc in range(KC_C):
                    pt = pj_pts[mc]
                    nc.vector.scalar_tensor_tensor(s2[:, mc, :], search_sb[:, mc, :],
                                                   1.0, pt[:, :, 0:288], ALU.mult, ALU.add)
                    nc.vector.tensor_scalar(y1[:, mc, :], s2[:, mc, :], THRESH, None,
                                            ALU.is_gt)
                y2 = sb.tile([128, KC_E, N], F32R, name="y2")
                for mcg in range(0, KC_E, 3):
                    f1_pts = [psm.tile([128, 2, 512], F32, tag=f"pw{i}", name=f"f1_ps{i}")
                              for i in range(3)]
                    first = True
                    for kc in range(KC_C):
                        for hl in ('hi', 'lo'):
                            wtl = load_w(wkp, f'wfc1_{hl}', kc, "w_big")
                            for i in range(3):
                                mc = mcg + i
                                for ih, (n0, nsz) in enumerate(NHS):
                                    nc.tensor.matmul(f1_pts[i][:, ih, :nsz],
                                                     wtl[:, mc * 128:(mc + 1) * 128],
                                                     y1[:, kc, n0:n0 + nsz],
                                                     start=first,
                                                     stop=(kc == KC_C - 1 and hl == 'lo'))
                            first = False
                    for i in range(3):
                        nc.vector.tensor_scalar(y2[:, mcg + i, :],
                                                f1_pts[i][:, :, 0:288], THRESH,
                                                None, ALU.is_gt)
                f2_pts = [psm.tile([128, 2, 512], F32, tag=f"pw{mc}", name=f"f2_ps{mc}")
                          for mc in range(KC_C)]
                first = True
                for kc in range(KC_E):
                    for hl in ('hi', 'lo'):
                        wtl = load_w(wkp, f'wfc2_{hl}', kc, "w_small")
                        for mc in range(KC_C):
                            for ih, (n0, nsz) in enumerate(NHS):
                                nc.tensor.matmul(f2_pts[mc][:, ih, :nsz],
                                                 wtl[:, mc * 128:(mc + 1) * 128],
                                                 y2[:, kc, n0:n0 + nsz],
                                                 start=first,
                                                 stop=(kc == KC_E - 1 and hl == 'lo'))
                        first = False
                for mc in range(KC_C):
                    pt = f2_pts[mc]
                    out_sb = wkp.tile([128, N], F32, tag="out_sb", name="out_sb")
                    nc.vector.scalar_tensor_tensor(out_sb[:], s2[:, mc, :], 1.0,
                                                   pt[:, :, 0:288], ALU.mult, ALU.add)
                    nc.sync.dma_start(
                        out_d[b].rearrange("(k p) n -> p k n", p=128)[:, mc, :],
                        out_sb[:])

    nc.compile()
    return nc


def kernel(templates, search, params):
    global _BUILT
    templates = np.asarray(templates, np.float32)
    search = np.asarray(search, np.float32)
    p = {k: np.asarray(v) for k, v in params.items()}
    if _BUILT is None:
        _BUILT = _build()
    nc = _BUILT
    wd = _prep_weights(p)
    tpos = np.asarray(p['t_pos'], np.float32).reshape(T, 1, DIM, N)
    spos = np.asarray(p['s_pos'], np.float32).reshape(1, DIM, N)
    tmpl_pre = templates.reshape(T, 16, DIM, N) + tpos       # [T,16,DIM,N]
    sr_pre = search.reshape(1, 16, DIM, N)[0] + spos         # [16,DIM,N]
    in_maps = []
    for core in range(8):
        m = dict(wd)
        m['templates'] = np.ascontiguousarray(tmpl_pre[:, 2 * core:2 * core + 2])
        m['search_pos'] = np.ascontiguousarray(sr_pre[2 * core:2 * core + 2])
        m['search'] = np.ascontiguousarray(
            search[0, 2 * core:2 * core + 2].reshape(NB, DIM, N))
        m['zeros16'] = np.zeros((16, N), np.float16)
        in_maps.append(m)
    r = run_bass_kernel_spmd(nc, in_maps, core_ids=list(range(8)))
    kernel.last_results = r
    out = np.stack([r.results[c]['out'] for c in range(8)])
    return out.reshape(1, 16, DIM, PATCH, PATCH).astype(np.float32)
